# revision 9
# baseline (speedup 1.0000x reference)
"""Multi-head attention Bass/Tile kernel for Trainium2, 8-core SPMD.

Problem: B=4, Q=K=2048, D=512, H=8 heads (head dim 64), fp32.
  head_q = q @ Wq.T ; head_k = k @ Wk.T ; head_v = v @ Wv.T
  S = (head_q . head_k) / 8 ; masked softmax over keys ; out = (P . head_v) @ Wo.T

Sharding: data-parallel over (batch, query-half): core c handles batch c//2,
query rows (c%2)*1024 .. +1024.  Each core computes a disjoint output slice;
no collectives.

v2 design (from the v1 trace: PE 78% busy with 30% of it transposes, ACT
exp 87us serializing the attention inner loop, HAM half-clock windows):
  - All transposes happen host-side: q/k/v arrive d-major ([D, rows]) and
    the weights arrive contraction-major ([D_in, D_out]).  No PE transposes,
    no identity matmuls, no weight-shift DMAs.
  - Masked kv rows are dropped host-side (softmax over keys is order-
    invariant; fully-masked rows contribute exactly 0).  KLE=1152 is the
    static padded key count (>5 sigma above Binomial(2048,.5)); kernel()
    compiles a KLE=2048 fallback on demand if a mask ever exceeds it.
  - Padded key rows are killed inside the exp: the activation bias is a
    per-(partition, key-tile) table holding 0 for real rows and -60 for
    padding, so e = exp(s/8 - 60) ~ 9e-27 vanishes.  V needs no masking,
    and the VS ones-column yields the softmax denominator exactly.
  - Scores/projections of q,k run in fp32r (1 cycle/row at free>=256);
    the V path, probabilities, and output projection run in bf16.
  - Per (ic, head-pair): two [64]x[128,512] score matmuls into one
    [128,1024] PSUM tile, one wide exp (ACT), two PV accumulations with a
    65th denominator column.  Normalization: DVE divide produces 1/denom
    rows packed in r8, a tiny K=2 selector matmul broadcasts them to 128
    partitions, and one DVE multiply per head evacuates PV directly.
  - Head-pair packed output projection: A2[hp] holds two heads on 128
    partitions, so the out-proj contracts over the full 128 partitions.
  - Projections are interleaved with attention per head-pair so ACT (exp)
    overlaps PE (next block's projections) and the PE never idles long
    enough for HAM to re-throttle.
"""

import sys

if "/opt/trn_rl_repo" not in sys.path:
    sys.path.insert(0, "/opt/trn_rl_repo")

from contextlib import ExitStack

import numpy as np

import concourse.bass as bass
import concourse.tile as tile
from concourse import mybir
import bass_rust as _bass_rust

F32 = mybir.dt.float32
F32R = mybir.dt.float32r
BF16 = mybir.dt.bfloat16
EXP = mybir.ActivationFunctionType.Exp
CPY = mybir.ActivationFunctionType.Copy

B, Q, KL, D, H = 4, 2048, 2048, 512, 8
HD = D // H            # 64
QS = Q // 2            # 1024 query rows per core
KLE_DEFAULT = 1152     # static padded bound on unmasked rows per batch
SCALE = 1.0 / HD ** 0.5
MASKBIAS = -60.0       # exp(s/8 - 60) ~ 1e-26: kills padded rows exactly enough


def _legalize_waits(nc, max_waits=1):
    """This walrus build only encodes one sem-wait per instruction; Tile's
    tail drain carries several.  Split extras onto preceding NoOps."""
    n = 0
    for f in nc.m.functions:
        for bb in f.blocks:
            insts = bb.instructions
            i = 0
            while i < len(insts):
                inst = insts[i]
                si = inst.sync_info
                if si is not None and len(si.on_wait) > max_waits:
                    waits = list(si.on_wait)
                    for j, w in enumerate(waits[max_waits:]):
                        nop = mybir.InstNoOp(
                            name=f"{inst.name}-waitsplit{j}", ins=[], outs=[]
                        )
                        nop.engine = inst.engine
                        nop.sync_info = _bass_rust.SyncInfo(on_wait=[w], on_update=[])
                        insts.insert(i, nop)
                        i += 1
                        n += 1
                    inst.sync_info = _bass_rust.SyncInfo(
                        on_wait=waits[:max_waits], on_update=list(si.on_update)
                    )
                i += 1
    return n


def _r(ap):
    return ap.bitcast(F32R)


def build_kernel(KLE=KLE_DEFAULT):
    NJT = KLE // 128
    nc = bass.Bass("TRN2", target_bir_lowering=False, debug=False)

    qT_d = nc.dram_tensor("qT", [D, QS], F32R, kind="ExternalInput").ap()
    kT_d = nc.dram_tensor("kT", [D, KLE], F32R, kind="ExternalInput").ap()
    vT_d = nc.dram_tensor("vT", [D, KLE], BF16, kind="ExternalInput").ap()
    wqT_d = nc.dram_tensor("wqT", [D, D], F32R, kind="ExternalInput").ap()
    wkT_d = nc.dram_tensor("wkT", [D, D], F32R, kind="ExternalInput").ap()
    wvT_d = nc.dram_tensor("wvT", [D, D], BF16, kind="ExternalInput").ap()
    woT_d = nc.dram_tensor("woT", [D, D], BF16, kind="ExternalInput").ap()
    eb_d = nc.dram_tensor("ebias", [128, NJT], F32, kind="ExternalInput").ap()
    out_d = nc.dram_tensor("out", [QS, D], F32, kind="ExternalOutput").ap()

    with tile.TileContext(nc) as tc, ExitStack() as ctx:
        # ---- persistent pools -------------------------------------------
        pc = ctx.enter_context(tc.tile_pool(name="const", bufs=1))
        # Band masks for the K=1 broadcast matmuls: cols 0:128 select the
        # even head's partition band, cols 128:256 the odd head's.
        sel = pc.tile([1, 256], BF16, tag="sel")
        nc.vector.memset(sel[:], 0.0)
        nc.vector.memset(sel[0:1, 0:64], 1.0)
        nc.vector.memset(sel[0:1, 192:256], 1.0)
        eb = pc.tile([128, NJT], F32, tag="eb")
        nc.sync.dma_start(eb[:], eb_d)

        pw = ctx.enter_context(tc.tile_pool(name="weights", bufs=1))
        wq = [pw.tile([128, D], F32R, tag=f"wq{i}", name=f"wq{i}") for i in range(4)]
        wk = [pw.tile([128, D], F32R, tag=f"wk{i}", name=f"wk{i}") for i in range(4)]
        wv = [pw.tile([128, D], BF16, tag=f"wv{i}", name=f"wv{i}") for i in range(4)]
        wo = [pw.tile([128, D], BF16, tag=f"wo{i}", name=f"wo{i}") for i in range(4)]

        pr = ctx.enter_context(tc.tile_pool(name="raw", bufs=1))
        kTr = [pr.tile([128, KLE], F32R, tag=f"kTr{i}", name=f"kTr{i}") for i in range(4)]
        qTr = [pr.tile([128, QS], F32R, tag=f"qTr{i}", name=f"qTr{i}") for i in range(4)]
        vTr = [pr.tile([128, KLE], BF16, tag=f"vTr{i}", name=f"vTr{i}") for i in range(4)]

        pp = ctx.enter_context(tc.tile_pool(name="proj", bufs=1))
        KT = [pp.tile([128, KLE], F32R, tag=f"KT{i}", name=f"KT{i}") for i in range(4)]
        QT = [pp.tile([128, QS], F32R, tag=f"QT{i}", name=f"QT{i}") for i in range(4)]
        VS = [pp.tile([128, H * (HD + 1)], BF16, tag=f"VS{i}", name=f"VS{i}") for i in range(NJT)]
        A2 = [pp.tile([128, QS], BF16, tag=f"A2{i}", name=f"A2{i}") for i in range(4)]

        # ---- DMA loads (priority order: k/q weights+acts first) ---------
        for dt_ in range(4):
            nc.sync.dma_start(wk[dt_][:], wkT_d.rearrange("(t p) d -> t p d", p=128)[dt_])
            nc.sync.dma_start(kTr[dt_][:], kT_d.rearrange("(t p) j -> t p j", p=128)[dt_])
        for dt_ in range(4):
            nc.sync.dma_start(wq[dt_][:], wqT_d.rearrange("(t p) d -> t p d", p=128)[dt_])
            nc.sync.dma_start(qTr[dt_][:], qT_d.rearrange("(t p) i -> t p i", p=128)[dt_])
        for dt_ in range(4):
            nc.sync.dma_start(wv[dt_][:], wvT_d.rearrange("(t p) d -> t p d", p=128)[dt_])
            nc.sync.dma_start(vTr[dt_][:], vT_d.rearrange("(t p) j -> t p j", p=128)[dt_])
        for dt_ in range(4):
            nc.sync.dma_start(wo[dt_][:], woT_d.rearrange("(t p) d -> t p d", p=128)[dt_])

        # K-chunks: equal 384-wide pieces keep fp32r at 1 cycle/row (>=256)
        kch = [(j0, min(384, KLE - j0)) for j0 in range(0, KLE, 384)]

        pe_t = ctx.enter_context(tc.tile_pool(name="et", bufs=1))
        po = ctx.enter_context(tc.tile_pool(name="osb", bufs=1))

        with tc.tile_pool(name="ps_sc", bufs=1, space="PSUM") as psc, \
             tc.tile_pool(name="ps_b", bufs=1, space="PSUM") as psb:

            def kproj(ot):
                for j0, jw in kch:
                    ps = psb.tile([128, 512], F32, tag="b512", bufs=4, name=f"kp{ot}_{j0}")
                    for dk in range(4):
                        nc.tensor.matmul(
                            ps[:, 0:jw],
                            wk[dk][:, ot * 128:(ot + 1) * 128],
                            kTr[dk][:, j0:j0 + jw],
                            start=(dk == 0), stop=(dk == 3),
                        )
                    nc.vector.tensor_copy(KT[ot][:, j0:j0 + jw], ps[:, 0:jw])

            def qproj(ot):
                for ic in range(2):
                    ps = psb.tile([128, 512], F32, tag="b512", bufs=4, name=f"qp{ot}_{ic}")
                    for dk in range(4):
                        nc.tensor.matmul(
                            ps[:],
                            wq[dk][:, ot * 128:(ot + 1) * 128],
                            qTr[dk][:, ic * 512:(ic + 1) * 512],
                            start=(dk == 0), stop=(dk == 3),
                        )
                    nc.vector.tensor_copy(QT[ot][:, ic * 512:(ic + 1) * 512], ps[:])

            def vproj(jt):
                ps = psb.tile([128, 512], F32, tag="b512", bufs=4, name=f"vp{jt}")
                for dk in range(4):
                    nc.tensor.matmul(
                        ps[:],
                        vTr[dk][:, jt * 128:(jt + 1) * 128],
                        wv[dk][:],
                        start=(dk == 0), stop=(dk == 3),
                    )
                vs3 = VS[jt][:].rearrange("p (h d) -> p h d", d=HD + 1)
                nc.vector.tensor_copy(vs3[:, :, 0:HD], ps[:].rearrange("p (h d) -> p h d", d=HD))
                nc.vector.memset(vs3[:, :, HD].squeeze(), 1.0)

            def attention(ic, hp):
                i0 = ic * 512
                he, ho = 2 * hp, 2 * hp + 1
                pv_e = psb.tile([65, 512], F32, tag="b512", bufs=4, name=f"pve{hp}_{ic}")
                pv_o = psb.tile([65, 512], F32, tag="b512", bufs=4, name=f"pvo{hp}_{ic}")
                ets = []
                for jt in range(NJT):
                    s_ps = psc.tile([128, 1024], F32, tag="sc", bufs=2, name=f"s{hp}_{ic}_{jt}")
                    for po2, sl in ((0, slice(0, 512)), (HD, slice(512, 1024))):
                        nc.tensor.matmul(
                            s_ps[:, sl],
                            KT[hp][po2:po2 + HD, jt * 128:(jt + 1) * 128],
                            QT[hp][po2:po2 + HD, i0:i0 + 512],
                            start=True, stop=True,
                        )
                    e_t = pe_t.tile([128, 1024], BF16, tag="e", bufs=4, name=f"e{hp}_{ic}_{jt}")
                    nc.scalar.activation(e_t[:], s_ps[:], EXP, scale=SCALE,
                                         bias=eb[:, jt:jt + 1])
                    ets.append(e_t)
                    # one-step software pipeline: PV trails scores by one tile
                    if jt > 0:
                        pv_step(pv_e, pv_o, ets[jt - 1], he, ho, jt - 1, NJT)
                pv_step(pv_e, pv_o, ets[NJT - 1], he, ho, NJT - 1, NJT)
                # normalization: r = 1/denominator per head, then two K=1
                # selector matmuls broadcast r to the head's partition band.
                r_e = pe_t.tile([1, 512], BF16, tag="r2", bufs=4, name=f"re{hp}_{ic}")
                r_o = pe_t.tile([1, 512], BF16, tag="r2", bufs=4, name=f"ro{hp}_{ic}")
                with nc.allow_low_precision(reason="1/denom in bf16; tol 2e-2"):
                    nc.vector.reciprocal(r_e[0:1, :], pv_e[64:65, :])
                    nc.vector.reciprocal(r_o[0:1, :], pv_o[64:65, :])
                bc = psb.tile([128, 512], F32, tag="b512", bufs=4, name=f"bc{hp}_{ic}")
                nc.tensor.matmul(
                    bc[:], sel[0:1, 0:128], r_e[0:1, :],
                    start=True, stop=False,
                )
                nc.tensor.matmul(
                    bc[:], sel[0:1, 128:256], r_o[0:1, :],
                    start=False, stop=True,
                )
                bc_sb = pe_t.tile([128, 512], BF16, tag="bcs", bufs=2, name=f"bcs{hp}_{ic}")
                nc.vector.tensor_copy(bc_sb[:], bc[:])
                nc.vector.tensor_mul(A2[hp][0:HD, i0:i0 + 512], pv_e[0:HD, :], bc_sb[0:HD, :])
                nc.vector.tensor_mul(A2[hp][HD:128, i0:i0 + 512], pv_o[0:HD, :], bc_sb[HD:128, :])

            def pv_step(pv_e, pv_o, e_t, he, ho, jt, njt):
                nc.tensor.matmul(
                    pv_e[:],
                    VS[jt][:, he * (HD + 1):(he + 1) * (HD + 1)],
                    e_t[:, 0:512],
                    start=(jt == 0), stop=(jt == njt - 1),
                )
                nc.tensor.matmul(
                    pv_o[:],
                    VS[jt][:, ho * (HD + 1):(ho + 1) * (HD + 1)],
                    e_t[:, 512:1024],
                    start=(jt == 0), stop=(jt == njt - 1),
                )

            def outproj(ic):
                for it in range(4):
                    c0 = ic * 512 + it * 128
                    o_ps = psb.tile([128, D], F32, tag="b512", bufs=4, name=f"op{ic}_{it}")
                    for hp in range(4):
                        nc.tensor.matmul(
                            o_ps[:],
                            A2[hp][:, c0:c0 + 128],
                            wo[hp][:],
                            start=(hp == 0), stop=(hp == 3),
                        )
                    o_sb = po.tile([128, D], F32, tag="osb", bufs=3, name=f"osb{ic}_{it}")
                    nc.vector.tensor_copy(o_sb[:], o_ps[:])
                    nc.sync.dma_start(out_d[c0:c0 + 128, :], o_sb[:])

            # ---- schedule ----------------------------------------------
            kproj(0)
            qproj(0)
            for jt in range(NJT):
                vproj(jt)
            for hp in range(4):
                if hp > 0:
                    kproj(hp)
                    qproj(hp)
                attention(0, hp)
                attention(1, hp)
                if hp == 3:
                    outproj(0)
                    outproj(1)

    return nc


_NC_CACHE = {}


def _get_nc(KLE):
    if KLE not in _NC_CACHE:
        nc = build_kernel(KLE)
        _legalize_waits(nc)
        _NC_CACHE[KLE] = nc
    return _NC_CACHE[KLE]


def shard_inputs(query, key, value, Wq, Wk, Wv, Wo, attn_mask, KLE=KLE_DEFAULT):
    """Per-core shards.  Masked kv rows are dropped (order-invariant under
    softmax; fully-masked rows contribute exactly 0), the rest packed into a
    static KLE-row buffer; padding is killed via the exp-bias table.  All
    activations/weights are pre-transposed to contraction-major layout."""
    import ml_dtypes

    bf16 = ml_dtypes.bfloat16
    NJT = KLE // 128
    wqT = np.ascontiguousarray(np.asarray(Wq, np.float32).T)
    wkT = np.ascontiguousarray(np.asarray(Wk, np.float32).T)
    wvT = np.ascontiguousarray(np.asarray(Wv, np.float32).T).astype(bf16)
    woT = np.ascontiguousarray(np.asarray(Wo, np.float32).T).astype(bf16)
    in_maps = []
    for c in range(8):
        b, half = c // 2, c % 2
        m = np.asarray(attn_mask[b]) != 0
        idx = np.nonzero(m)[0]
        n = len(idx)
        if n > KLE:
            raise ValueError(f"unmasked count {n} exceeds KLE={KLE}")
        kT = np.zeros((D, KLE), np.float32)
        vT = np.zeros((D, KLE), np.float32)
        kT[:, :n] = np.asarray(key[b], np.float32)[idx].T
        vT[:, :n] = np.asarray(value[b], np.float32)[idx].T
        ebias = np.zeros((128, NJT), np.float32)
        flat = (np.arange(KLE) >= n).reshape(NJT, 128).T  # [p, jt] padded?
        ebias[flat] = MASKBIAS
        in_maps.append({
            "qT": np.ascontiguousarray(
                np.asarray(query[b, half * QS:(half + 1) * QS], np.float32).T),
            "kT": kT,
            "vT": vT.astype(bf16),
            "wqT": wqT, "wkT": wkT, "wvT": wvT, "woT": woT,
            "ebias": ebias,
        })
    return in_maps


def kernel(query, key, value, Wq, Wk, Wv, Wo, attn_mask, _trace=False, _trace_kwargs=None):
    from concourse.bass_utils import run_bass_kernel_spmd

    counts = [(np.asarray(attn_mask[b]) != 0).sum() for b in range(B)]
    KLE = KLE_DEFAULT if max(counts) <= KLE_DEFAULT else KL
    in_maps = shard_inputs(query, key, value, Wq, Wk, Wv, Wo, attn_mask, KLE)
    nc = _get_nc(KLE)
    res = run_bass_kernel_spmd(
        nc, in_maps, list(range(8)), trace=_trace, **(_trace_kwargs or {})
    )
    out = np.empty((B, Q, D), dtype=np.float32)
    for c in range(8):
        b, half = c // 2, c % 2
        out[b, half * QS:(half + 1) * QS] = res.results[c]["out"]
    if _trace:
        kernel._last_results = res
    return out
